# revision 10
# baseline (speedup 1.0000x reference)
"""GCN (2x shared GCNConv+BN+LeakyReLU, linear head) on 8 trn2 NeuronCores.

Nodes row-sharded 8 ways (12500/core, 98 tiles of 128). All feature math in
bf16 with fp32 PSUM accumulation. Structure per core:

  z1 = x @ (W1@Wc) for OWN rows only -> table1 (raw z rows, no pre-scales).
  Halo z rows (remote in-neighbors) arrive via two pipelined AllToAlls per
  layer (chunk a = rows first used by dst tiles < T_HALF, chunk b = rest);
  each collective writes straight into the gather table (Shared DRAM), and
  fills are checkpoint-pipelined during the producing z/conv phase.
  Aggregation is transposed one-hot matmuls: stationary = gathered z rows
  (slot-major), moving = one-hot whose entries are dinv[src]*dinv[dst]
  (host-baked), so agg lands feature-major in PSUM with the symmetric
  normalization fully applied -- no vector post-scales, no PE transposes.
  Self-loops are a contiguous DMA block + host diagonal one-hot (dinv^2).
  BN stats: one bn_stats per (group, chunk) PSUM slab, bn_aggr per layer,
  4KB AllReduce of (sum, sumsq). BN+LeakyReLU is one scalar-engine
  activation per (group, chunk); conv z2 = h1 @ Wc feeds table2 and the
  layer-2 fills. Output head uses folded W2@WO. Edge gathers round-robin
  over 4 SWDGE queues (gathers are row-rate limited).
"""

import math
import os

if os.environ.get("AXON_LOOPBACK_RELAY") or os.environ.get("AXON_POOL_SVC_OVERRIDE"):
    _jp = os.environ.get("JAX_PLATFORMS")
    if _jp and "axon" not in _jp:
        os.environ["JAX_PLATFORMS"] = "axon," + _jp

import numpy as np
import ml_dtypes

from concourse import bacc, bass, mybir, tile
from concourse.bass_utils import run_bass_kernel_spmd

BF16 = mybir.dt.bfloat16
F32 = mybir.dt.float32
I16 = mybir.dt.int16
NP_BF16 = ml_dtypes.bfloat16

P = 128
EPS = 1e-5
ALPHA = 0.01
GRP = 4
T_HALF = 48          # dst tiles < T_HALF only need halo chunk a
CKPT_GROUP = 12      # z/conv group count before fill checkpoint 0


def _wrap_idx(flat):
    n = len(flat)
    assert n % 16 == 0
    w = np.zeros((16, n // 16), np.int16)
    w[np.arange(n) % 16, np.arange(n) // 16] = flat.astype(np.int16)
    return np.ascontiguousarray(np.tile(w, (8, 1)))


def _ceil16(n):
    return (n + 15) // 16 * 16


# ---------------------------------------------------------------------------
# Host-side planning
# ---------------------------------------------------------------------------

def make_plan(x, edge_index, W1, b1, Wc, bc, gamma, beta, W2, b2, WO, bO, C=8):
    x = np.asarray(x, np.float32)
    ei = np.asarray(edge_index).astype(np.int64)
    src, dst = ei[0], ei[1]
    N, F = x.shape
    H = np.asarray(Wc).shape[0]
    CH = H // P
    S = N // C
    T_OWN = math.ceil(S / P)
    OWN_PAD = T_OWN * P
    NG = math.ceil(T_OWN / GRP)
    CKPT_ROWS = CKPT_GROUP * GRP * P

    deg = np.bincount(dst, minlength=N).astype(np.float64) + 1.0
    dinv = 1.0 / np.sqrt(deg)

    owner_s = src // S
    owner_d = dst // S

    # ---- halo sets: UA/UB[j][k] = srcs in j needed by k, chunked by the
    # receiver's first-use tile, each sorted by src index, and split into
    # (rows < CKPT_ROWS locally, rest) 16-padded segments.
    UA = [[None] * C for _ in range(C)]
    UB = [[None] * C for _ in range(C)]
    for k in range(C):
        m = owner_d == k
        es, ed = src[m], dst[m] - k * S
        for j in range(C):
            if j == k:
                continue
            mj = owner_s[m] == j
            u, inv = np.unique(es[mj], return_inverse=True)
            ft = np.full(len(u), 1 << 30)
            np.minimum.at(ft, inv, ed[mj] // P)
            UA[j][k] = np.sort(u[ft < T_HALF])
            UB[j][k] = np.sort(u[ft >= T_HALF])

    # per (chunk, sender j, receiver k): seg0 = rows with local idx < CKPT_ROWS
    def seg_sizes(U):
        n0 = np.zeros((C, C), np.int64)
        n1 = np.zeros((C, C), np.int64)
        for j in range(C):
            for k in range(C):
                if j == k:
                    continue
                loc = U[j][k] - j * S
                n0[j, k] = int(np.searchsorted(loc, CKPT_ROWS))
                n1[j, k] = len(loc)
        return n0, n1

    nA0, nA1 = seg_sizes(UA)
    nB0, nB1 = seg_sizes(UB)
    # shared program segment sizes per sender-slot: max over (j,k) pairs
    A_SEG0 = _ceil16(int(nA0.max()))
    A_SEG1 = _ceil16(int((nA1 - nA0).max()))
    B_SEG0 = _ceil16(int(nB0.max()))
    B_SEG1 = _ceil16(int((nB1 - nB0).max()))
    R_A = A_SEG0 + A_SEG1
    R_B = B_SEG0 + B_SEG1
    RECV_A = OWN_PAD
    RECV_B = OWN_PAD + C * R_A
    TABLE_ROWS = OWN_PAD + C * (R_A + R_B)
    assert TABLE_ROWS <= 32767, TABLE_ROWS

    # table position of halo src u for pair (j -> k)
    def table_pos(j, k, u):
        ua, ub = UA[j][k], UB[j][k]
        loc = u - j * S
        pa = np.searchsorted(ua, u)
        ina = np.zeros(len(u), bool)
        if len(ua):
            ina = (pa < len(ua)) & (ua[np.minimum(pa, len(ua) - 1)] == u)
        pos = np.empty(len(u), np.int64)
        # chunk a
        la = ua - j * S
        m0 = int(np.searchsorted(la, CKPT_ROWS))
        ra = pa[ina]
        pos[ina] = RECV_A + j * R_A + np.where(ra < m0, ra, A_SEG0 + (ra - m0))
        # chunk b
        ub_ = ub
        lb = ub_ - j * S
        m0b = int(np.searchsorted(lb, CKPT_ROWS))
        pb = np.searchsorted(ub_, u[~ina])
        pos[~ina] = RECV_B + j * R_B + np.where(pb < m0b, pb,
                                                B_SEG0 + (pb - m0b))
        return pos

    # ---- folded weights
    W1 = np.asarray(W1, np.float64)
    Wc64 = np.asarray(Wc, np.float64)
    Wf = (W1 @ Wc64).astype(np.float32)
    bf_row = (np.asarray(b1, np.float64) @ Wc64).astype(np.float32)
    WfO = (np.asarray(W2, np.float64) @ np.asarray(WO, np.float64)).astype(np.float32)
    bOf = float(np.asarray(b2, np.float64) @ np.asarray(WO, np.float64)[:, 0]
                + np.asarray(bO, np.float64)[0])
    has_bias = bool(np.any(bf_row != 0.0))

    def _wlayout(W):
        return np.ascontiguousarray(
            W.reshape(CH, P, H).transpose(1, 0, 2).reshape(P, CH * H).astype(NP_BF16))

    wf_host = _wlayout(Wf)
    wc_host = _wlayout(np.asarray(Wc, np.float32))
    wfo_host = np.ascontiguousarray(WfO.reshape(CH, P).T.astype(NP_BF16))
    gmb_host = np.concatenate(
        [np.asarray(gamma, np.float32).reshape(CH, P).T,
         np.asarray(beta, np.float32).reshape(CH, P).T], axis=1)
    brow_host = bf_row.reshape(1, H).astype(NP_BF16)
    onesrow_host = np.ones((1, P), dtype=NP_BF16)

    # ---- per-core edge lists and unified block structure
    per_core = []
    blocks_max = np.ones(T_OWN, np.int64)
    for k in range(C):
        m = owner_d == k
        es, ed = src[m], dst[m] - k * S
        order = np.argsort(ed, kind="stable")
        es, ed = es[order], ed[order]
        cnt = np.bincount(ed // P, minlength=T_OWN)
        blocks_max = np.maximum(blocks_max,
                                np.ceil(cnt / P).astype(np.int64))
        per_core.append((es, ed))
    blk_off = np.zeros(T_OWN + 1, np.int64)
    np.cumsum(blocks_max, out=blk_off[1:])
    TOT_BLK = int(blk_off[-1])

    in_maps = []
    for k in range(C):
        es, ed = per_core[k]
        row = np.zeros(len(es), np.int64)
        mloc = (es // S) == k
        row[mloc] = es[mloc] - k * S
        for j in range(C):
            if j == k:
                continue
            mj = (es // S) == j
            if mj.any():
                row[mj] = table_pos(j, k, es[mj])

        tile_id = ed // P
        starts = np.searchsorted(tile_id, np.arange(T_OWN))
        r_in_tile = np.arange(len(ed)) - starts[tile_id]
        gflat = np.zeros(TOT_BLK * P, np.int64)
        gflat[blk_off[tile_id] * P + r_in_tile] = row
        gidx = _wrap_idx(gflat)

        ohE = np.zeros((P, TOT_BLK * P), NP_BF16)
        val = (dinv[es] * dinv[ed + k * S]).astype(np.float32)
        ohE[r_in_tile % P,
            (blk_off[tile_id] + r_in_tile // P) * P + (ed % P)] = val

        ohS = np.zeros((P, T_OWN * P), NP_BF16)
        own_ids = np.arange(S)
        ohS[own_ids % P, own_ids] = (dinv[k * S:(k + 1) * S] ** 2).astype(np.float32)

        # fill index arrays (k as sender): per chunk, per receiver slot j
        # (self slot zeroed): [seg0-rows pad to SEG0, seg1-rows pad to SEG1]
        def fill_flat(U, SEG0, SEG1):
            cols = []
            for j in range(C):
                s0 = np.zeros(SEG0, np.int64)
                s1 = np.zeros(SEG1, np.int64)
                if j != k:
                    loc = U[k][j] - k * S
                    m0 = int(np.searchsorted(loc, CKPT_ROWS))
                    s0[:m0] = loc[:m0]
                    s1[:len(loc) - m0] = loc[m0:]
                cols.append(np.concatenate([s0, s1]))
            return _wrap_idx(np.concatenate(cols))

        agidx_a = fill_flat(UA, A_SEG0, A_SEG1)
        agidx_b = fill_flat(UB, B_SEG0, B_SEG1)

        xo = np.zeros((F, OWN_PAD), NP_BF16)
        xo[:, :S] = x[k * S:(k + 1) * S].T

        in_maps.append({
            "xt_own": xo, "gidx": gidx, "ohE": ohE, "ohS": ohS,
            "agidx_a": agidx_a, "agidx_b": agidx_b,
            "wf": wf_host, "wc": wc_host, "wfo": wfo_host, "gmb": gmb_host,
            "brow": brow_host, "onesrow": onesrow_host,
        })

    dims = dict(N=N, H=H, CH=CH, C=C, S=S, T_OWN=T_OWN, OWN_PAD=OWN_PAD,
                NG=NG, R_A=R_A, R_B=R_B, RECV_A=RECV_A, RECV_B=RECV_B,
                A_SEG0=A_SEG0, A_SEG1=A_SEG1, B_SEG0=B_SEG0, B_SEG1=B_SEG1,
                TABLE_ROWS=TABLE_ROWS, TOT_BLK=TOT_BLK,
                blk_off=tuple(int(v) for v in blk_off),
                bOf=bOf, has_bias=has_bias)
    return dims, in_maps


# ---------------------------------------------------------------------------
# Device program
# ---------------------------------------------------------------------------

def build_program(d):
    C, H, CH = d["C"], d["H"], d["CH"]
    S, T_OWN, OWN_PAD, NG = d["S"], d["T_OWN"], d["OWN_PAD"], d["NG"]
    R_A, R_B = d["R_A"], d["R_B"]
    RECV_A, RECV_B = d["RECV_A"], d["RECV_B"]
    A_SEG0, A_SEG1 = d["A_SEG0"], d["A_SEG1"]
    B_SEG0, B_SEG1 = d["B_SEG0"], d["B_SEG1"]
    TABLE_ROWS, TOT_BLK = d["TABLE_ROWS"], d["TOT_BLK"]
    blk_off = d["blk_off"]
    N = d["N"]
    CKPT_ROWS = CKPT_GROUP * GRP * P
    groups = [list(range(C))]
    Lrelu = mybir.ActivationFunctionType.Lrelu
    Sqrt = mybir.ActivationFunctionType.Sqrt
    Sigmoid = mybir.ActivationFunctionType.Sigmoid
    Copy = mybir.ActivationFunctionType.Copy
    Add = mybir.AluOpType.add

    FA_COLS = C * R_A // 16
    FB_COLS = C * R_B // 16

    nc = bacc.Bacc("TRN2", target_bir_lowering=False, debug=False,
                   enable_asserts=False, num_devices=C, num_swdge_queues=4)

    xt_own_d = nc.dram_tensor("xt_own", [H, OWN_PAD], BF16, kind="ExternalInput")
    gidx_d = nc.dram_tensor("gidx", [P, TOT_BLK * P // 16], I16, kind="ExternalInput")
    agidx_a_d = nc.dram_tensor("agidx_a", [P, FA_COLS], I16, kind="ExternalInput")
    agidx_b_d = nc.dram_tensor("agidx_b", [P, FB_COLS], I16, kind="ExternalInput")
    ohE_d = nc.dram_tensor("ohE", [P, TOT_BLK * P], BF16, kind="ExternalInput")
    ohS_d = nc.dram_tensor("ohS", [P, T_OWN * P], BF16, kind="ExternalInput")
    wf_d = nc.dram_tensor("wf", [P, CH * H], BF16, kind="ExternalInput")
    wc_d = nc.dram_tensor("wc", [P, CH * H], BF16, kind="ExternalInput")
    wfo_d = nc.dram_tensor("wfo", [P, CH], BF16, kind="ExternalInput")
    gmb_d = nc.dram_tensor("gmb", [P, 2 * CH], F32, kind="ExternalInput")
    brow_d = nc.dram_tensor("brow", [1, H], BF16, kind="ExternalInput")
    onesrow_d = nc.dram_tensor("onesrow", [1, P], BF16, kind="ExternalInput")
    out_ext = nc.dram_tensor("out", [S, 1], F32, kind="ExternalOutput")

    def cdiv(a, b):
        return (a + b - 1) // b

    with tile.TileContext(nc) as tc:
        with (
            tc.tile_pool(name="consts", bufs=1) as cp,
            tc.tile_pool(name="work", bufs=2) as wp,
            tc.tile_pool(name="psum", bufs=1, space="PSUM") as pp,
            tc.tile_pool(name="dram", bufs=1, space="DRAM") as dp,
        ):
            # ---- constants
            gidx_sb = cp.tile([P, TOT_BLK * P // 16], I16, name="gidx_sb")
            nc.sync.dma_start(out=gidx_sb, in_=gidx_d[:, :])
            agidx_a_sb = cp.tile([P, FA_COLS], I16, name="agidx_a_sb")
            nc.sync.dma_start(out=agidx_a_sb, in_=agidx_a_d[:, :])
            agidx_b_sb = cp.tile([P, FB_COLS], I16, name="agidx_b_sb")
            nc.sync.dma_start(out=agidx_b_sb, in_=agidx_b_d[:, :])
            wf_sb = cp.tile([P, CH * H], BF16, name="wf_sb")
            nc.sync.dma_start(out=wf_sb, in_=wf_d[:, :])
            wc_sb = cp.tile([P, CH * H], BF16, name="wc_sb")
            nc.sync.dma_start(out=wc_sb, in_=wc_d[:, :])
            wfo_sb = cp.tile([P, CH], BF16, name="wfo_sb")
            nc.sync.dma_start(out=wfo_sb, in_=wfo_d[:, :])
            gmb_sb = cp.tile([P, 2 * CH], F32, name="gmb_sb")
            nc.sync.dma_start(out=gmb_sb, in_=gmb_d[:, :])
            brow_sb = cp.tile([1, H], BF16, name="brow_sb")
            nc.sync.dma_start(out=brow_sb, in_=brow_d[:, :])
            onesrow_sb = cp.tile([1, P], BF16, name="onesrow_sb")
            nc.sync.dma_start(out=onesrow_sb, in_=onesrow_d[:, :])

            aggT_sb = cp.tile([P, NG * CH * GRP * P], BF16, name="aggT_sb")
            stats_sb = [cp.tile([P, NG * CH * 8], F32, name=f"stats{l}")
                        for l in range(2)]
            outcols = cp.tile([P, T_OWN], F32, name="outcols")

            # ---- DRAM internals
            table1 = dp.tile([TABLE_ROWS, H], BF16, name="table1")
            table2 = dp.tile([TABLE_ROWS, H], BF16, name="table2")
            a2a_in_a = dp.tile([C * R_A, H], BF16, name="a2a_in_a")
            a2a_in_b = dp.tile([C * R_B, H], BF16, name="a2a_in_b")
            ar_in = [dp.tile([P, 2 * CH], F32, name=f"ar_in{l}")
                     for l in range(2)]
            ar_out = [dp.tile([P, 2 * CH], F32, addr_space="Shared",
                              name=f"ar_out{l}") for l in range(2)]

            qrr = [0]

            def next_q():
                q = qrr[0]
                qrr[0] = (qrr[0] + 1) % 4
                return q

            # ================= fills =================
            # fill arrays have C slots (self slot zeroed) so slot id == peer
            # id in both the idx array and the a2a input buffer.
            def fill2(table, a2a_in, aidx_sb, R_c, SEG0, SEG1, ckpt, tag):
                bound = CKPT_ROWS if ckpt == 0 else OWN_PAD
                seg_base, seg_len = (0, SEG0) if ckpt == 0 else (SEG0, SEG1)
                if seg_len == 0:
                    return
                for sl in range(C):
                    flat0 = sl * R_c + seg_base
                    ni = seg_len
                    gg = wp.tile([P, cdiv(ni, P) * H], BF16, tag="gg",
                                 bufs=2, name=f"gg_{tag}_{ckpt}_{sl}")
                    for c0 in range(0, ni, 1024):
                        cn = min(1024, ni - c0)
                        i0 = flat0 + c0
                        nc.gpsimd.dma_gather(
                            out_ap=gg.rearrange("p (b h) -> p b h", h=H)[
                                :, c0 // P:c0 // P + cdiv(cn, P), :],
                            in_ap=table[0:bound, :],
                            idxs_ap=aidx_sb[:, i0 // 16:(i0 + cn) // 16],
                            num_idxs=cn, num_idxs_reg=cn, elem_size=H,
                            queue_num=next_q())
                    base = sl * R_c + seg_base
                    full = ni // P
                    rem = ni - full * P
                    if full:
                        nc.sync.dma_start(
                            out=a2a_in[base:base + full * P, :].rearrange(
                                "(b p) h -> p b h", p=P),
                            in_=gg.rearrange("p (b h) -> p b h",
                                             h=H)[:, 0:full, :])
                    if rem:
                        nc.sync.dma_start(
                            out=a2a_in[base + full * P:base + ni, :].rearrange(
                                "(b p) h -> p b h", p=rem),
                            in_=gg.rearrange("p (b h) -> p b h",
                                             h=H)[0:rem, full:full + 1, :])

            # ================= z / conv phases =================
            def z_phase(dest_table, tag, producer):
                for g in range(NG):
                    g0 = g * GRP
                    gn = min(GRP, T_OWN - g0)
                    producer(g0, gn)
                    if g == CKPT_GROUP - 1:
                        fill2(dest_table, a2a_in_a, agidx_a_sb, R_A,
                              A_SEG0, A_SEG1, 0, f"fa{tag}")
                        fill2(dest_table, a2a_in_b, agidx_b_sb, R_B,
                              B_SEG0, B_SEG1, 0, f"fb{tag}")
                fill2(dest_table, a2a_in_a, agidx_a_sb, R_A, A_SEG0, A_SEG1,
                      1, f"fa{tag}")
                nc.gpsimd.collective_compute(
                    "AllToAll", mybir.AluOpType.bypass, replica_groups=groups,
                    ins=[a2a_in_a.opt()],
                    outs=[dest_table[RECV_A:RECV_A + C * R_A, :]])
                fill2(dest_table, a2a_in_b, agidx_b_sb, R_B, B_SEG0, B_SEG1,
                      1, f"fb{tag}")
                nc.gpsimd.collective_compute(
                    "AllToAll", mybir.AluOpType.bypass, replica_groups=groups,
                    ins=[a2a_in_b.opt()],
                    outs=[dest_table[RECV_B:RECV_B + C * R_B, :]])

            def z1_producer(g0, gn):
                xt4 = wp.tile([P, gn * H], BF16, tag="xt4", bufs=2,
                              name=f"xt4_{g0}")
                nc.sync.dma_start(
                    out=xt4.rearrange("p (c q n) -> p c q n", q=gn, n=P),
                    in_=xt_own_d[:, g0 * P:(g0 + gn) * P].rearrange(
                        "(c p) (q n) -> p c q n", p=P, n=P))
                zh4 = wp.tile([P, gn * H], BF16, tag="zh4", bufs=2,
                              name=f"zh4_{g0}")
                for q in range(gn):
                    zp = pp.tile([P, H], F32, tag="zp", bufs=3,
                                 name=f"zp_{g0}_{q}")
                    for c in range(CH):
                        last = (c == CH - 1) and not d["has_bias"]
                        nc.tensor.matmul(
                            zp,
                            lhsT=xt4[:, (c * gn + q) * P:(c * gn + q + 1) * P],
                            rhs=wf_sb[:, c * H:(c + 1) * H],
                            start=(c == 0), stop=last)
                    if d["has_bias"]:
                        nc.tensor.matmul(zp, lhsT=onesrow_sb[:, :],
                                         rhs=brow_sb[:, :], start=False,
                                         stop=True)
                    nc.scalar.activation(zh4[:, q * H:(q + 1) * H], zp, Copy)
                nc.sync.dma_start(
                    out=table1[g0 * P:(g0 + gn) * P, :].rearrange(
                        "(q p) h -> p q h", p=P),
                    in_=zh4.rearrange("p (q h) -> p q h", h=H))

            # ================= aggregation =================
            def agg_phase(table, l, b_start_grp):
                stats = stats_sb[l]
                for g in range(NG):
                    g0 = g * GRP
                    gn = min(GRP, T_OWN - g0)
                    nb = blk_off[min(g0 + gn, T_OWN)] - blk_off[g0]
                    ohe = wp.tile([P, nb * P], BF16, tag="ohe", bufs=3,
                                  name=f"ohe_{l}_{g0}")
                    nc.sync.dma_start(
                        out=ohe,
                        in_=ohE_d[:, blk_off[g0] * P:blk_off[g0 + gn] * P])
                    ohs = wp.tile([P, gn * P], BF16, tag="ohs", bufs=3,
                                  name=f"ohs_{l}_{g0}")
                    nc.sync.dma_start(
                        out=ohs, in_=ohS_d[:, g0 * P:(g0 + gn) * P])
                    selfg = wp.tile([P, gn * H], BF16, tag="selfg", bufs=2,
                                    name=f"selfg_{l}_{g0}")
                    nc.sync.dma_start(
                        out=selfg.rearrange("p (q h) -> p q h", h=H),
                        in_=table[g0 * P:(g0 + gn) * P, :].rearrange(
                            "(q p) h -> p q h", p=P))
                    ni = nb * P
                    hi_bound = RECV_B if g < b_start_grp else TABLE_ROWS
                    g4 = wp.tile([P, nb * H], BF16, tag="g4", bufs=3,
                                 name=f"g4_{l}_{g0}")
                    assert ni <= 1024
                    nc.gpsimd.dma_gather(
                        out_ap=g4.rearrange("p (b h) -> p b h", h=H),
                        in_ap=table[0:hi_bound, :],
                        idxs_ap=gidx_sb[:, blk_off[g0] * P // 16:
                                        blk_off[g0 + gn] * P // 16],
                        num_idxs=ni, num_idxs_reg=ni, elem_size=H,
                        queue_num=next_q())
                    for c in range(CH):
                        pg = pp.tile([P, GRP * P], F32, tag="agg", bufs=4,
                                     name=f"agg_{l}_{g0}_{c}")
                        for q in range(gn):
                            t = g0 + q
                            nbt = blk_off[t + 1] - blk_off[t]
                            bb = blk_off[t] - blk_off[g0]
                            nc.tensor.matmul(
                                pg[:, q * P:(q + 1) * P],
                                lhsT=selfg[:, q * H + c * P:
                                           q * H + (c + 1) * P],
                                rhs=ohs[:, q * P:(q + 1) * P],
                                start=True, stop=(nbt == 0),
                                skip_group_check=True)
                            for b in range(nbt):
                                nc.tensor.matmul(
                                    pg[:, q * P:(q + 1) * P],
                                    lhsT=g4[:, (bb + b) * H + c * P:
                                            (bb + b) * H + (c + 1) * P],
                                    rhs=ohe[:, (bb + b) * P:(bb + b + 1) * P],
                                    start=False, stop=(b == nbt - 1),
                                    skip_group_check=True)
                        nc.vector.bn_stats(
                            stats[:, (c * NG + g) * 8:(c * NG + g) * 8 + 6],
                            pg[:, 0:gn * P])
                        nc.scalar.activation(
                            aggT_sb[:, (g * CH + c) * GRP * P:
                                    (g * CH + c) * GRP * P + gn * P],
                            pg[:, 0:gn * P], Copy)

            # ================= BN coeffs =================
            def bn_coeffs(l):
                stats = stats_sb[l]
                mv = cp.tile([P, 2 * CH], F32, name=f"mv_{l}")
                for c in range(CH):
                    nc.vector.bn_aggr(
                        mv[:, 2 * c:2 * c + 2],
                        stats[:, c * NG * 8:(c + 1) * NG * 8].rearrange(
                            "p (g s) -> p g s", s=8)[:, :, 0:6])
                mu_l = mv.rearrange("p (c two) -> p c two", two=2)[:, :, 0]
                var_l = mv.rearrange("p (c two) -> p c two", two=2)[:, :, 1]
                sums = cp.tile([P, 2 * CH], F32, name=f"sums_{l}")
                m2 = cp.tile([P, CH], F32, name=f"m2_{l}")
                nc.vector.tensor_mul(m2, mu_l, mu_l)
                nc.vector.tensor_add(m2, var_l, m2)
                nc.vector.tensor_scalar_mul(sums[:, 0:CH], mu_l, float(OWN_PAD))
                nc.vector.tensor_scalar_mul(sums[:, CH:2 * CH], m2,
                                            float(OWN_PAD))
                nc.sync.dma_start(out=ar_in[l][:, :], in_=sums)
                nc.gpsimd.collective_compute(
                    "AllReduce", Add, replica_groups=groups,
                    ins=[ar_in[l].opt()], outs=[ar_out[l].opt()])
                sg = cp.tile([P, 2 * CH], F32, name=f"sg_{l}")
                nc.sync.dma_start(out=sg, in_=ar_out[l][:, :])
                mu = cp.tile([P, CH], F32, name=f"mu_{l}")
                nc.vector.tensor_scalar_mul(mu, sg[:, 0:CH], 1.0 / N)
                ex2 = cp.tile([P, CH], F32, name=f"ex2_{l}")
                nc.vector.tensor_scalar_mul(ex2, sg[:, CH:2 * CH], 1.0 / N)
                var = cp.tile([P, CH], F32, name=f"var_{l}")
                nc.vector.tensor_mul(var, mu, mu)
                nc.vector.tensor_sub(var, ex2, var)
                nc.vector.tensor_scalar_add(var, var, EPS)
                std = cp.tile([P, CH], F32, name=f"std_{l}")
                nc.scalar.activation(std, var, Sqrt)
                rstd = cp.tile([P, CH], F32, name=f"rstd_{l}")
                nc.vector.reciprocal(rstd, std)
                scale = cp.tile([P, CH], F32, name=f"scale_{l}")
                nc.vector.tensor_mul(scale, gmb_sb[:, 0:CH], rstd)
                shift = cp.tile([P, CH], F32, name=f"shift_{l}")
                nc.vector.tensor_mul(shift, mu, scale)
                nc.vector.tensor_sub(shift, gmb_sb[:, CH:2 * CH], shift)
                return scale, shift

            # ================= conv / head =================
            def conv_producer_factory(scale, shift):
                def conv_producer(g0, gn):
                    g = g0 // GRP
                    ht = wp.tile([P, CH * gn * P], BF16, tag="ht", bufs=2,
                                 name=f"ht_{g0}")
                    for c in range(CH):
                        nc.scalar.activation(
                            ht[:, c * gn * P:(c + 1) * gn * P],
                            aggT_sb[:, (g * CH + c) * GRP * P:
                                    (g * CH + c) * GRP * P + gn * P],
                            Lrelu, bias=shift[:, c:c + 1],
                            scale=scale[:, c:c + 1], alpha=ALPHA)
                    zh4 = wp.tile([P, gn * H], BF16, tag="zh4", bufs=2,
                                  name=f"zh4c_{g0}")
                    for q in range(gn):
                        zp = pp.tile([P, H], F32, tag="zp", bufs=3,
                                     name=f"zp2_{g0}_{q}")
                        for c in range(CH):
                            nc.tensor.matmul(
                                zp,
                                lhsT=ht[:, (c * gn + q) * P:
                                        (c * gn + q + 1) * P],
                                rhs=wc_sb[:, c * H:(c + 1) * H],
                                start=(c == 0), stop=(c == CH - 1))
                        nc.vector.tensor_copy(zh4[:, q * H:(q + 1) * H], zp)
                    nc.sync.dma_start(
                        out=table2[g0 * P:(g0 + gn) * P, :].rearrange(
                            "(q p) h -> p q h", p=P),
                        in_=zh4.rearrange("p (q h) -> p q h", h=H))
                return conv_producer

            def head(scale, shift):
                for g in range(NG):
                    g0 = g * GRP
                    gn = min(GRP, T_OWN - g0)
                    ht = wp.tile([P, CH * gn * P], BF16, tag="ht", bufs=2,
                                 name=f"hto_{g0}")
                    for c in range(CH):
                        nc.scalar.activation(
                            ht[:, c * gn * P:(c + 1) * gn * P],
                            aggT_sb[:, (g * CH + c) * GRP * P:
                                    (g * CH + c) * GRP * P + gn * P],
                            Lrelu, bias=shift[:, c:c + 1],
                            scale=scale[:, c:c + 1], alpha=ALPHA)
                    for q in range(gn):
                        t = g0 + q
                        op = pp.tile([P, 1], F32, tag="op", bufs=1,
                                     name=f"op_{t}")
                        for c in range(CH):
                            nc.tensor.matmul(
                                op, lhsT=ht[:, (c * gn + q) * P:
                                            (c * gn + q + 1) * P],
                                rhs=wfo_sb[:, c:c + 1],
                                start=(c == 0), stop=(c == CH - 1),
                                skip_group_check=True)
                        nc.vector.tensor_copy(outcols[:, t:t + 1], op)

            # ================= pipeline =================
            B_START_GRP = T_HALF // GRP
            z_phase(table1, "1", z1_producer)
            agg_phase(table1, 0, B_START_GRP)
            scale1, shift1 = bn_coeffs(0)
            z_phase(table2, "2", conv_producer_factory(scale1, shift1))
            agg_phase(table2, 1, B_START_GRP)
            scale2, shift2 = bn_coeffs(1)
            head(scale2, shift2)

            sig = cp.tile([P, T_OWN], F32, name="sig")
            nc.scalar.activation(sig, outcols, Sigmoid, bias=float(d["bOf"]),
                                 scale=1.0)
            full_t = S // P
            rem = S - full_t * P
            if full_t:
                nc.sync.dma_start(
                    out=out_ext[0:full_t * P, :].rearrange(
                        "(t p) one -> p (t one)", p=P),
                    in_=sig[:, 0:full_t])
            if rem:
                nc.sync.dma_start(
                    out=out_ext[full_t * P:S, :].rearrange(
                        "(q p) h -> p q h", p=rem),
                    in_=sig[0:rem, full_t:full_t + 1].rearrange(
                        "p (q h) -> p q h", q=1))

    nc.compile()
    return nc


# ---------------------------------------------------------------------------
# Entry point
# ---------------------------------------------------------------------------

_CACHE = {}


def _get_program(dims):
    key = tuple(sorted((k, str(v)) for k, v in dims.items()))
    if key not in _CACHE:
        _CACHE[key] = build_program(dims)
    return _CACHE[key]


def kernel(x, edge_index, W1, b1, Wc, bc, gamma, beta, W2, b2, WO, bO,
           trace=False):
    dims, in_maps = make_plan(x, edge_index, W1, b1, Wc, bc, gamma, beta,
                              W2, b2, WO, bO)
    nc = _get_program(dims)
    res = run_bass_kernel_spmd(nc, in_maps, core_ids=list(range(dims["C"])),
                               trace=trace)
    out = np.concatenate([r["out"] for r in res.results], axis=0)
    kernel.last_results = res
    return out.astype(np.float32)
